# revision 25
# baseline (speedup 1.0000x reference)
"""GatedGCN layer as a Bass/Tile kernel for 8 Trainium2 NeuronCores.

Sharding: data-parallel over the batch dim B=8 (one batch element per
core); graph structure and weights replicated. BatchNorm moments are
combined with an 8-core AllReduce.

Math (per batch element b):
  x    = X[b] @ W1                                  [N,C]
  y    = x @ (v * w2row)      (w2 folded into v)    [N,C]
  aggr = segsum_tgt(weff[e] * y[src[e]])            [N,C]   weff = w/cnt
  out  = x @ u + aggr
  BN over (batch, channel) per node, then x + relu(out_norm)

Device mapping per core:
  - XbT [C,N] shipped transposed in bf16; per node-tile (128 nodes) one
    bf16 matmul with moving [W1|W1v2] produces x and y tiles in [n,d]
    layout; y is written to HBM (bf16 rows) as the gather table.
  - Edges are sorted by target on the host and padded per node-tile to
    blocks of 128. SWDGE dma_gather (<=1024 rows per instruction, the
    hardware descriptor-ring limit) pulls y[src] rows into SBUF
    [128 edges x 128 ch]; per 128-edge block a selection matrix
    M[e,n] = weff[e]*(tgt_local[e]==n) is built with one DVE tensor_scalar
    and matmul'd on the PE, accumulating z + sum_j M_j.T @ Y_j in PSUM.
  - BN stats via ACT accum_out, AllReduce over a DRAM bounce buffer, then
    whole-array normalize/relu/residual with broadcast APs, batched DMA out.
"""

import numpy as np

N = 10000
C = 128
E = 160000
B = 8
EPS = 1e-5

NT = (N + 127) // 128  # 79 node tiles
NPAD = NT * 128        # 10112

_cache = {}


def _prep_graph(edge_index, edge_weight):
    """Sort edges by target, pad per node-tile to blocks of 128, build
    gather-index / target-local / effective-weight arrays in the layouts
    the device consumes."""
    import ml_dtypes

    src = np.asarray(edge_index[0], dtype=np.int64)
    tgt = np.asarray(edge_index[1], dtype=np.int64)
    w = np.asarray(edge_weight, dtype=np.float32)

    counts = np.bincount(tgt, minlength=N).astype(np.float32)
    inv = 1.0 / np.maximum(counts, 1.0)

    order = np.argsort(tgt, kind="stable")
    src_s = src[order]
    tgt_s = tgt[order]
    weff_s = (w[order] * inv[tgt_s]).astype(np.float32)

    tile_of = tgt_s >> 7
    starts = np.searchsorted(tile_of, np.arange(NT))
    ends = np.searchsorted(tile_of, np.arange(NT) + 1)

    nblks = []
    gidx_parts = []   # [16, S] int16 pieces
    tgtl_parts = []   # [128, nblk] bf16 pieces
    weff_parts = []
    for t in range(NT):
        s, e = int(starts[t]), int(ends[t])
        cnt = e - s
        nblk = (cnt + 127) // 128
        nblks.append(nblk)
        if nblk == 0:
            continue
        pad = nblk * 128 - cnt
        idxs = np.concatenate([src_s[s:e], np.zeros(pad, np.int64)]).astype(np.int16)
        tl = np.concatenate([tgt_s[s:e] - 128 * t, np.zeros(pad, np.int64)])
        wf = np.concatenate([weff_s[s:e], np.zeros(pad, np.float32)])
        # gather wrap: element i read from [i % 16, i // 16]
        gidx_parts.append(idxs.reshape(-1, 16).T)
        # block layout: edge slot i -> (partition i % 128, block i // 128)
        tgtl_parts.append(tl.reshape(nblk, 128).T.astype(ml_dtypes.bfloat16))
        weff_parts.append(wf.reshape(nblk, 128).T.astype(ml_dtypes.bfloat16))

    gidx = np.concatenate(gidx_parts, axis=1).copy()  # [16, NBLK*8]
    tgtl = np.concatenate(tgtl_parts, axis=1).copy()                   # [128, NBLK]
    weff = np.concatenate(weff_parts, axis=1).copy()
    return nblks, gidx, tgtl, weff


def build_program(n_cores, nblks, nbl_total, npad, nt, stats_scale):
    import concourse.bass as bass
    import concourse.bacc as bacc
    import concourse.tile as tile
    import concourse.mybir as mybir
    from concourse import library_config

    f32 = mybir.dt.float32
    bf16 = mybir.dt.bfloat16
    i16 = mybir.dt.int16
    AF = mybir.ActivationFunctionType
    OP = mybir.AluOpType

    nc = bacc.Bacc("TRN2", target_bir_lowering=False, debug=False,
                   num_devices=n_cores, num_swdge_queues=2)

    xbt_d = nc.dram_tensor("XBT", [C, npad], bf16, kind="ExternalInput")
    w12_d = nc.dram_tensor("W12", [C, 2 * C], bf16, kind="ExternalInput")
    w1uu_d = nc.dram_tensor("W1UU", [C, 2 * C], bf16, kind="ExternalInput")
    gidx_d = nc.dram_tensor("GIDX", [16, nbl_total * 8], i16, kind="ExternalInput")
    tgtl_d = nc.dram_tensor("TGTL", [128, nbl_total], bf16, kind="ExternalInput")
    weff_d = nc.dram_tensor("WEFF", [128, nbl_total], bf16, kind="ExternalInput")
    colx_d = nc.dram_tensor("COLX", [128, 128], bf16, kind="ExternalInput")
    out_d = nc.dram_tensor("OUT", [npad, C], bf16, kind="ExternalOutput")

    with tile.TileContext(nc) as tc:
        with (
            tc.tile_pool(name="big", bufs=1) as big,
            tc.tile_pool(name="dram", bufs=1, space="DRAM") as dram,
            tc.tile_pool(name="work", bufs=3) as work,
            tc.tile_pool(name="gath", bufs=8) as gpool,
            tc.tile_pool(name="mmat", bufs=6) as mpool,
            tc.tile_pool(name="psa", bufs=4, space="PSUM") as psa,
            tc.tile_pool(name="psb", bufs=4, space="PSUM") as psb,
        ):
            nc.gpsimd.load_library(library_config.mlp)

            # --- static SBUF loads -------------------------------------
            xbt = big.tile([C, npad], bf16, tag="xbt")
            w12 = big.tile([C, 2 * C], bf16, tag="w12")
            w1uu = big.tile([C, 2 * C], bf16, tag="w1uu")
            gidx = big.tile([128, nbl_total * 8], i16, tag="gidx")
            tgtlb = big.tile([128, nbl_total], bf16, tag="tgtlb")
            weffb = big.tile([128, nbl_total], bf16, tag="weffb")
            tgtl = big.tile([128, nbl_total], f32, tag="tgtl")
            weff = big.tile([128, nbl_total], f32, tag="weff")
            x_sb = big.tile([128, nt * 128], bf16, tag="x")
            o_sb = big.tile([128, nt * 128], f32, tag="o")
            obf = big.tile([128, nt * 128], bf16, tag="obf")
            stats = big.tile([128, 2 * nt], f32, tag="stats")
            ssum = big.tile([128, 2 * nt], f32, tag="ssum")
            colx = big.tile([128, 128], bf16, tag="colx")

            nc.sync.dma_start(xbt[:], xbt_d.ap())
            nc.sync.dma_start(w12[:], w12_d.ap())
            nc.sync.dma_start(w1uu[:], w1uu_d.ap())
            for k in range(8):
                nc.sync.dma_start(gidx[16 * k:16 * (k + 1), :], gidx_d.ap())
            nc.sync.dma_start(tgtlb[:], tgtl_d.ap())
            nc.sync.dma_start(weffb[:], weff_d.ap())
            nc.vector.tensor_copy(tgtl[:], tgtlb[:])
            nc.vector.tensor_copy(weff[:], weffb[:])
            nc.sync.dma_start(colx[:], colx_d.ap())

            y_dram = dram.tile([npad, C], bf16)

            # --- phase A: x, y per node tile; y -> HBM (batched) -------
            YB = 8
            ybuf = None
            for t in range(nt):
                ps = psa.tile([128, 2 * C], f32, tag="psa")
                nc.tensor.matmul(
                    ps[:],
                    xbt[:, t * 128:(t + 1) * 128],
                    w12[:],
                    start=True, stop=True,
                )
                if t % 2 == 0:
                    nc.scalar.copy(x_sb[:, t * 128:(t + 1) * 128], ps[:, 0:C])
                else:
                    nc.vector.tensor_copy(
                        x_sb[:, t * 128:(t + 1) * 128], ps[:, 0:C])
                if t % YB == 0:
                    ybuf = work.tile([128, YB, C], bf16, tag="ybuf")
                    yb0 = t
                nc.vector.tensor_copy(ybuf[:, t - yb0, :], ps[:, C:2 * C])
                if t == nt - 1 or t - yb0 == YB - 1:
                    nb = t - yb0 + 1
                    dst = y_dram[yb0 * 128:(t + 1) * 128, :]
                    nc.sync.dma_start(
                        dst.rearrange("(j p) c -> p j c", p=128),
                        ybuf[:, 0:nb, :])

            # BN finalize for node-tiles [h0, h1): AllReduce the per-node
            # moments (elementwise per node, so freely splittable), compute
            # istd/bias, normalize+relu+residual, store. Emitted per half so
            # the first half overlaps the remaining gather window.
            mean = big.tile([128, nt], f32, tag="mean")
            istd = big.tile([128, nt], f32, tag="istd")
            nbias = big.tile([128, nt], f32, tag="nbias")
            btmp = big.tile([128, nt], f32, tag="btmp")

            def finalize(h0, h1, hi):
                hn = h1 - h0
                if n_cores > 1:
                    b_in = dram.tile([128, 2 * hn], f32, tag=f"bin{hi}")
                    b_out = dram.tile([128, 2 * hn], f32, tag=f"bout{hi}")
                    nc.sync.dma_start(b_in[:, 0:hn], stats[:, h0:h1])
                    nc.sync.dma_start(b_in[:, hn:2 * hn],
                                      stats[:, nt + h0:nt + h1])
                    nc.gpsimd.collective_compute(
                        "AllReduce",
                        mybir.AluOpType.add,
                        replica_groups=[list(range(n_cores))],
                        ins=[b_in.opt()],
                        outs=[b_out.opt()],
                    )
                    nc.sync.dma_start(ssum[:, h0:h1], b_out[:, 0:hn])
                    nc.sync.dma_start(ssum[:, nt + h0:nt + h1],
                                      b_out[:, hn:2 * hn])
                    st = ssum
                else:
                    st = stats
                me, isd, nb, tp = (mean[:, h0:h1], istd[:, h0:h1],
                                   nbias[:, h0:h1], btmp[:, h0:h1])
                nc.vector.tensor_scalar_mul(me, st[:, h0:h1], stats_scale)
                nc.vector.tensor_scalar_mul(tp, st[:, nt + h0:nt + h1],
                                            stats_scale)
                nc.vector.tensor_tensor(nb, me, me, op=OP.mult)
                nc.vector.tensor_tensor(tp, tp, nb, op=OP.subtract)
                nc.vector.tensor_scalar_add(tp, tp, EPS)
                # istd = exp(-0.5 * ln(var + eps))
                nc.scalar.activation(tp, tp, AF.Ln)
                nc.scalar.activation(isd, tp, AF.Exp, scale=-0.5)
                nc.vector.tensor_tensor(tp, me, isd, op=OP.mult)
                nc.vector.tensor_scalar_mul(nb, tp, -1.0)

                o3 = o_sb[:].rearrange("p (t n) -> p t n", n=128)[:, h0:h1, :]
                x3 = x_sb[:].rearrange("p (t n) -> p t n", n=128)[:, h0:h1, :]
                ob3 = obf[:].rearrange("p (t n) -> p t n", n=128)[:, h0:h1, :]
                isd_b = isd.unsqueeze(2).broadcast_to([128, hn, 128])
                nb_b = nb.unsqueeze(2).broadcast_to([128, hn, 128])
                nc.vector.tensor_tensor(o3, o3, isd_b, op=OP.mult)
                nc.vector.tensor_tensor(o3, o3, nb_b, op=OP.add)
                osl = o_sb[:, h0 * 128:h1 * 128]
                nc.scalar.activation(osl, osl, AF.Relu)
                OB = 8
                ob_full = obf[:].rearrange("p (t n) -> p t n", n=128)
                for t0 in range(h0, h1, OB):
                    tb = min(OB, h1 - t0)
                    nc.vector.tensor_tensor(
                        ob_full[:, t0:t0 + tb, :],
                        o_sb[:].rearrange("p (t n) -> p t n", n=128)
                        [:, t0:t0 + tb, :],
                        x_sb[:].rearrange("p (t n) -> p t n", n=128)
                        [:, t0:t0 + tb, :], op=OP.add)
                    dst = out_d.ap()[t0 * 128:(t0 + tb) * 128, :]
                    nc.sync.dma_start(
                        dst.rearrange("(j p) c -> p j c", p=128),
                        ob_full[:, t0:t0 + tb, :])

            if nt >= 16:
                qs = [nt // 4, nt // 2, 3 * nt // 4, nt]
            else:
                qs = [nt]

            # --- phase B: gather + selection matmuls per node tile -----
            # Gathers are flat 8-block (1024-row, the SWDGE ring limit)
            # chunks over the global block list, decoupled from tile
            # boundaries, so no gather instruction runs partially filled.
            CH = 8
            chunk_tiles = {}
            next_chunk = [0]

            def ensure_chunks(upto_blk):
                while next_chunk[0] * CH < upto_blk:
                    k = next_chunk[0]
                    cb = min(CH, nbl_total - k * CH)
                    g = gpool.tile([128, CH, C], bf16, tag="gath")
                    nc.gpsimd.dma_gather(
                        g[:, 0:cb, :],
                        y_dram[:],
                        gidx[:, k * CH * 8:(k * CH + cb) * 8],
                        cb * 128,
                        cb * 128,
                        C,
                        queue_num=k % 2,
                    )
                    chunk_tiles[k] = g
                    next_chunk[0] += 1

            blk0 = 0
            for t in range(nt):
                nblk = nblks[t]
                ps = psb.tile([128, 2 * C], f32, tag="psb")
                nc.tensor.matmul(
                    ps[:],
                    xbt[:, t * 128:(t + 1) * 128],
                    w1uu[:],
                    start=True, stop=(nblk == 0),
                )
                if nblk > 0:
                    ensure_chunks(blk0 + nblk)
                    for j in range(nblk):
                        b = blk0 + j
                        m = mpool.tile([128, 128], bf16, tag="m")
                        nc.vector.tensor_scalar(
                            m[:], colx[:],
                            tgtl[:, b:b + 1],
                            weff[:, b:b + 1],
                            op0=OP.is_equal, op1=OP.mult,
                        )
                        nc.tensor.matmul(
                            ps[:, 0:C], m[:],
                            chunk_tiles[b // CH][:, b % CH, :],
                            start=False, stop=(j == nblk - 1),
                        )
                    blk0 += nblk
                # out tile copy + moments
                nc.scalar.activation(
                    o_sb[:, t * 128:(t + 1) * 128], ps[:, 0:C], AF.Copy,
                    accum_out=stats[:, t:t + 1],
                )
                sq = work.tile([128, C], f32, tag="sq")
                nc.scalar.activation(
                    sq[:], ps[:, 0:C], AF.Square,
                    accum_out=stats[:, nt + t:nt + t + 1],
                )
                for qi in range(len(qs) - 1):
                    if t == qs[qi] + 6:
                        finalize(qs[qi - 1] if qi else 0, qs[qi], qi)

            finalize(qs[-2] if len(qs) > 1 else 0, qs[-1], len(qs) - 1)

    nc.compile()
    return nc


def _get_compiled(key, n_cores, nblks, nbl_total, npad, nt, stats_scale):
    if key not in _cache:
        _cache[key] = build_program(n_cores, nblks, nbl_total, npad, nt,
                                    stats_scale)
    return _cache[key]


LAST_EXEC_NS = None
LAST_RESULT = None
LAST_DISPATCH = None  # (disp, dev_in) of the most recent fast-path call


def _fast_dispatch(nc, n_cores):
    """Build (once) a cached jit'd shard_map dispatcher for the compiled
    Bass module, mirroring bass2jax.run_bass_via_pjrt but with on-device
    output buffers and reusable device-resident inputs."""
    import jax
    import jax.numpy as jnp
    import concourse.mybir as mybir
    from jax.experimental.shard_map import shard_map
    from jax.sharding import Mesh, PartitionSpec
    from concourse import bass2jax

    bass2jax.install_neuronx_cc_hook()

    in_names, out_names, out_avals = [], [], []
    partition_name = (nc.partition_id_tensor.name
                      if nc.partition_id_tensor else None)
    for alloc in nc.m.functions[0].allocations:
        if not isinstance(alloc, mybir.MemoryLocationSet):
            continue
        name = alloc.memorylocations[0].name
        if alloc.kind == "ExternalInput":
            if name != partition_name:
                in_names.append(name)
        elif alloc.kind == "ExternalOutput":
            out_names.append(name)
            out_avals.append(jax.core.ShapedArray(
                tuple(alloc.tensor_shape), mybir.dt.np(alloc.dtype)))
    n_params = len(in_names)
    all_in_names = list(in_names) + list(out_names)
    if partition_name is not None:
        all_in_names.append(partition_name)

    def _body(*args):
        operands = list(args)
        if partition_name is not None:
            operands.append(bass2jax.partition_id_tensor())
        outs = bass2jax._bass_exec_p.bind(
            *operands,
            out_avals=tuple(out_avals),
            in_names=tuple(all_in_names),
            out_names=tuple(out_names),
            lowering_input_output_aliases=(),
            sim_require_finite=True,
            sim_require_nnan=True,
            nc=nc,
        )
        return tuple(outs)

    devices = jax.devices()[:n_cores]
    mesh = Mesh(np.asarray(devices), ("core",))
    n_outs = len(out_avals)
    donate = tuple(range(n_params, n_params + n_outs))
    in_specs = (PartitionSpec("core"),) * (n_params + n_outs)
    out_specs = (PartitionSpec("core"),) * n_outs
    sharded = jax.jit(
        shard_map(_body, mesh=mesh, in_specs=in_specs, out_specs=out_specs,
                  check_rep=False),
        donate_argnums=donate, keep_unused=True)

    from jax.sharding import NamedSharding
    shard = NamedSharding(mesh, PartitionSpec("core"))

    def make_zeros():
        return [jax.device_put(
            jnp.zeros((n_cores * a.shape[0], *a.shape[1:]), a.dtype), shard)
            for a in out_avals]

    return {"sharded": sharded, "in_names": in_names,
            "out_names": out_names, "out_avals": out_avals,
            "shard": shard, "make_zeros": make_zeros, "n_cores": n_cores}


def _run_fast(disp, in_maps):
    global LAST_DISPATCH
    import jax
    import hashlib
    n_cores = disp["n_cores"]
    dev_in = []
    for i, name in enumerate(disp["in_names"]):
        arrs = [np.asarray(m[name]) for m in in_maps]
        h = hashlib.blake2b(digest_size=16)
        h.update(name.encode())
        for a in arrs:
            h.update(a.tobytes())
        key = ("devin", h.hexdigest())
        if key not in _cache:
            cat = np.concatenate(arrs, axis=0)
            _cache[key] = jax.device_put(cat, disp["shard"])
            _cache[key].block_until_ready()
        dev_in.append(_cache[key])
    LAST_DISPATCH = (disp, dev_in)
    zeros = disp["make_zeros"]()
    out_arrs = disp["sharded"](*dev_in, *zeros)
    out_arrs = [np.asarray(o) for o in out_arrs]
    return [
        {name: out_arrs[i].reshape(n_cores, *disp["out_avals"][i].shape)[c]
         for i, name in enumerate(disp["out_names"])}
        for c in range(n_cores)
    ]


def kernel(X, edge_index, edge_weight, weight1, weight2, u, v, _trace=False):
    global LAST_EXEC_NS, LAST_RESULT
    from concourse.bass_utils import run_bass_kernel_spmd

    X = np.asarray(X, dtype=np.float32)
    weight1 = np.asarray(weight1, dtype=np.float32)
    weight2 = np.asarray(weight2, dtype=np.float32)
    u = np.asarray(u, dtype=np.float32)
    v = np.asarray(v, dtype=np.float32)

    ek = (np.asarray(edge_index).tobytes(), np.asarray(edge_weight).tobytes())
    gk = hash(ek)
    if ("graph", gk) not in _cache:
        _cache[("graph", gk)] = _prep_graph(edge_index, edge_weight)
    nblks, gidx, tgtl, weff = _cache[("graph", gk)]
    nbl_total = sum(nblks)

    nc = _get_compiled(("prog", tuple(nblks)), B, nblks, nbl_total, NPAD, NT,
                       1.0 / (B * C))

    import ml_dtypes as mld
    w12 = np.concatenate([weight1, weight1 @ (v * weight2[0][None, :])],
                         axis=1).astype(mld.bfloat16)
    w1uu_half = (weight1 @ u).astype(np.float32)
    w1uu = np.concatenate([w1uu_half, w1uu_half], axis=1).astype(mld.bfloat16)

    xbt = np.zeros((B, C, NPAD), dtype=mld.bfloat16)
    xbt[:, :, :N] = X.transpose(0, 2, 1).astype(mld.bfloat16)

    colx = np.tile(np.arange(C, dtype=np.float32), (128, 1)).astype(mld.bfloat16)
    shared = {"W12": w12, "W1UU": w1uu, "GIDX": gidx, "TGTL": tgtl,
              "WEFF": weff, "COLX": colx}
    in_maps = [{"XBT": np.ascontiguousarray(xbt[i]), **shared}
               for i in range(B)]
    import os
    results = None
    if not _trace and not os.environ.get("KERNEL_NO_FAST"):
        try:
            dk = ("disp", id(nc))
            if dk not in _cache:
                _cache[dk] = _fast_dispatch(nc, B)
            results = _run_fast(_cache[dk], in_maps)
        except Exception:
            results = None
    if results is None:
        res = run_bass_kernel_spmd(nc, in_maps, core_ids=list(range(B)),
                                   trace=_trace)
        LAST_EXEC_NS = res.exec_time_ns
        LAST_RESULT = res
        results = res.results
    out = np.stack([results[i]["OUT"][:N].astype(np.float32)
                    for i in range(B)])
    return out


# revision 26
# speedup vs baseline: 1.0134x; 1.0134x over previous
"""GatedGCN layer as a Bass/Tile kernel for 8 Trainium2 NeuronCores.

Sharding: data-parallel over the batch dim B=8 (one batch element per
core); graph structure and weights replicated. BatchNorm moments are
combined with an 8-core AllReduce.

Math (per batch element b):
  x    = X[b] @ W1                                  [N,C]
  y    = x @ (v * w2row)      (w2 folded into v)    [N,C]
  aggr = segsum_tgt(weff[e] * y[src[e]])            [N,C]   weff = w/cnt
  out  = x @ u + aggr
  BN over (batch, channel) per node, then x + relu(out_norm)

Device mapping per core:
  - XbT [C,N] shipped transposed in bf16; per node-tile (128 nodes) one
    bf16 matmul with moving [W1|W1v2] produces x and y tiles in [n,d]
    layout; y is written to HBM (bf16 rows) as the gather table.
  - Edges are sorted by target on the host and padded per node-tile to
    blocks of 128. SWDGE dma_gather (<=1024 rows per instruction, the
    hardware descriptor-ring limit) pulls y[src] rows into SBUF
    [128 edges x 128 ch]; per 128-edge block a selection matrix
    M[e,n] = weff[e]*(tgt_local[e]==n) is built with one DVE tensor_scalar
    and matmul'd on the PE, accumulating z + sum_j M_j.T @ Y_j in PSUM.
  - BN stats via ACT accum_out, AllReduce over a DRAM bounce buffer, then
    whole-array normalize/relu/residual with broadcast APs, batched DMA out.
"""

import numpy as np

N = 10000
C = 128
E = 160000
B = 8
EPS = 1e-5

NT = (N + 127) // 128  # 79 node tiles
NPAD = NT * 128        # 10112

_cache = {}


def _prep_graph(edge_index, edge_weight):
    """Sort edges by target, pad per node-tile to blocks of 128, build
    gather-index / target-local / effective-weight arrays in the layouts
    the device consumes."""
    import ml_dtypes

    src = np.asarray(edge_index[0], dtype=np.int64)
    tgt = np.asarray(edge_index[1], dtype=np.int64)
    w = np.asarray(edge_weight, dtype=np.float32)

    counts = np.bincount(tgt, minlength=N).astype(np.float32)
    inv = 1.0 / np.maximum(counts, 1.0)

    order = np.argsort(tgt, kind="stable")
    src_s = src[order]
    tgt_s = tgt[order]
    weff_s = (w[order] * inv[tgt_s]).astype(np.float32)

    tile_of = tgt_s >> 7
    starts = np.searchsorted(tile_of, np.arange(NT))
    ends = np.searchsorted(tile_of, np.arange(NT) + 1)

    nblks = []
    gidx_parts = []   # [16, S] int16 pieces
    tgtl_parts = []   # [128, nblk] bf16 pieces
    weff_parts = []
    for t in range(NT):
        s, e = int(starts[t]), int(ends[t])
        cnt = e - s
        nblk = (cnt + 127) // 128
        nblks.append(nblk)
        if nblk == 0:
            continue
        pad = nblk * 128 - cnt
        idxs = np.concatenate([src_s[s:e], np.zeros(pad, np.int64)]).astype(np.int16)
        tl = np.concatenate([tgt_s[s:e] - 128 * t, np.zeros(pad, np.int64)])
        wf = np.concatenate([weff_s[s:e], np.zeros(pad, np.float32)])
        # gather wrap: element i read from [i % 16, i // 16]
        gidx_parts.append(idxs.reshape(-1, 16).T)
        # block layout: edge slot i -> (partition i % 128, block i // 128)
        tgtl_parts.append(tl.reshape(nblk, 128).T.astype(ml_dtypes.bfloat16))
        weff_parts.append(wf.reshape(nblk, 128).T.astype(ml_dtypes.bfloat16))

    gidx = np.concatenate(gidx_parts, axis=1).copy()  # [16, NBLK*8]
    tgtl = np.concatenate(tgtl_parts, axis=1).copy()                   # [128, NBLK]
    weff = np.concatenate(weff_parts, axis=1).copy()
    return nblks, gidx, tgtl, weff


def build_program(n_cores, nblks, nbl_total, npad, nt, stats_scale):
    import concourse.bass as bass
    import concourse.bacc as bacc
    import concourse.tile as tile
    import concourse.mybir as mybir
    from concourse import library_config

    f32 = mybir.dt.float32
    bf16 = mybir.dt.bfloat16
    i16 = mybir.dt.int16
    AF = mybir.ActivationFunctionType
    OP = mybir.AluOpType

    nc = bacc.Bacc("TRN2", target_bir_lowering=False, debug=False,
                   num_devices=n_cores, num_swdge_queues=2)

    xbt_d = nc.dram_tensor("XBT", [C, npad], bf16, kind="ExternalInput")
    w12_d = nc.dram_tensor("W12", [C, 2 * C], bf16, kind="ExternalInput")
    w1uu_d = nc.dram_tensor("W1UU", [C, 2 * C], bf16, kind="ExternalInput")
    gidx_d = nc.dram_tensor("GIDX", [16, nbl_total * 8], i16, kind="ExternalInput")
    tgtl_d = nc.dram_tensor("TGTL", [128, nbl_total], bf16, kind="ExternalInput")
    weff_d = nc.dram_tensor("WEFF", [128, nbl_total], bf16, kind="ExternalInput")
    colx_d = nc.dram_tensor("COLX", [128, 128], bf16, kind="ExternalInput")
    out_d = nc.dram_tensor("OUT", [npad, C], bf16, kind="ExternalOutput")

    with tile.TileContext(nc) as tc:
        with (
            tc.tile_pool(name="big", bufs=1) as big,
            tc.tile_pool(name="dram", bufs=1, space="DRAM") as dram,
            tc.tile_pool(name="work", bufs=3) as work,
            tc.tile_pool(name="gath", bufs=12) as gpool,
            tc.tile_pool(name="mmat", bufs=10) as mpool,
            tc.tile_pool(name="psa", bufs=4, space="PSUM") as psa,
            tc.tile_pool(name="psb", bufs=4, space="PSUM") as psb,
        ):
            nc.gpsimd.load_library(library_config.mlp)

            # --- static SBUF loads -------------------------------------
            xbt = big.tile([C, npad], bf16, tag="xbt")
            w12 = big.tile([C, 2 * C], bf16, tag="w12")
            w1uu = big.tile([C, 2 * C], bf16, tag="w1uu")
            gidx = big.tile([128, nbl_total * 8], i16, tag="gidx")
            tgtlb = big.tile([128, nbl_total], bf16, tag="tgtlb")
            weffb = big.tile([128, nbl_total], bf16, tag="weffb")
            tgtl = big.tile([128, nbl_total], f32, tag="tgtl")
            weff = big.tile([128, nbl_total], f32, tag="weff")
            x_sb = big.tile([128, nt * 128], bf16, tag="x")
            o_sb = big.tile([128, nt * 128], f32, tag="o")
            obf = big.tile([128, nt * 128], bf16, tag="obf")
            stats = big.tile([128, 2 * nt], f32, tag="stats")
            ssum = big.tile([128, 2 * nt], f32, tag="ssum")
            colx = big.tile([128, 128], bf16, tag="colx")

            nc.sync.dma_start(xbt[:], xbt_d.ap())
            nc.sync.dma_start(w12[:], w12_d.ap())
            nc.sync.dma_start(w1uu[:], w1uu_d.ap())
            for k in range(8):
                nc.sync.dma_start(gidx[16 * k:16 * (k + 1), :], gidx_d.ap())
            nc.sync.dma_start(tgtlb[:], tgtl_d.ap())
            nc.sync.dma_start(weffb[:], weff_d.ap())
            nc.vector.tensor_copy(tgtl[:], tgtlb[:])
            nc.vector.tensor_copy(weff[:], weffb[:])
            nc.sync.dma_start(colx[:], colx_d.ap())

            y_dram = dram.tile([npad, C], bf16)

            # --- phase A: x, y per node tile; y -> HBM (batched) -------
            YB = 8
            ybuf = None
            for t in range(nt):
                ps = psa.tile([128, 2 * C], f32, tag="psa")
                nc.tensor.matmul(
                    ps[:],
                    xbt[:, t * 128:(t + 1) * 128],
                    w12[:],
                    start=True, stop=True,
                )
                if t % 2 == 0:
                    nc.scalar.copy(x_sb[:, t * 128:(t + 1) * 128], ps[:, 0:C])
                else:
                    nc.vector.tensor_copy(
                        x_sb[:, t * 128:(t + 1) * 128], ps[:, 0:C])
                if t % YB == 0:
                    ybuf = work.tile([128, YB, C], bf16, tag="ybuf")
                    yb0 = t
                nc.vector.tensor_copy(ybuf[:, t - yb0, :], ps[:, C:2 * C])
                if t == nt - 1 or t - yb0 == YB - 1:
                    nb = t - yb0 + 1
                    dst = y_dram[yb0 * 128:(t + 1) * 128, :]
                    nc.sync.dma_start(
                        dst.rearrange("(j p) c -> p j c", p=128),
                        ybuf[:, 0:nb, :])

            # BN finalize for node-tiles [h0, h1): AllReduce the per-node
            # moments (elementwise per node, so freely splittable), compute
            # istd/bias, normalize+relu+residual, store. Emitted per half so
            # the first half overlaps the remaining gather window.
            mean = big.tile([128, nt], f32, tag="mean")
            istd = big.tile([128, nt], f32, tag="istd")
            nbias = big.tile([128, nt], f32, tag="nbias")
            btmp = big.tile([128, nt], f32, tag="btmp")

            def finalize(h0, h1, hi):
                hn = h1 - h0
                if n_cores > 1:
                    b_in = dram.tile([128, 2 * hn], f32, tag=f"bin{hi}")
                    b_out = dram.tile([128, 2 * hn], f32, tag=f"bout{hi}")
                    nc.sync.dma_start(b_in[:, 0:hn], stats[:, h0:h1])
                    nc.sync.dma_start(b_in[:, hn:2 * hn],
                                      stats[:, nt + h0:nt + h1])
                    nc.gpsimd.collective_compute(
                        "AllReduce",
                        mybir.AluOpType.add,
                        replica_groups=[list(range(n_cores))],
                        ins=[b_in.opt()],
                        outs=[b_out.opt()],
                    )
                    nc.sync.dma_start(ssum[:, h0:h1], b_out[:, 0:hn])
                    nc.sync.dma_start(ssum[:, nt + h0:nt + h1],
                                      b_out[:, hn:2 * hn])
                    st = ssum
                else:
                    st = stats
                me, isd, nb, tp = (mean[:, h0:h1], istd[:, h0:h1],
                                   nbias[:, h0:h1], btmp[:, h0:h1])
                nc.vector.tensor_scalar_mul(me, st[:, h0:h1], stats_scale)
                nc.vector.tensor_scalar_mul(tp, st[:, nt + h0:nt + h1],
                                            stats_scale)
                nc.vector.tensor_tensor(nb, me, me, op=OP.mult)
                nc.vector.tensor_tensor(tp, tp, nb, op=OP.subtract)
                nc.vector.tensor_scalar_add(tp, tp, EPS)
                # istd = exp(-0.5 * ln(var + eps))
                nc.scalar.activation(tp, tp, AF.Ln)
                nc.scalar.activation(isd, tp, AF.Exp, scale=-0.5)
                nc.vector.tensor_tensor(tp, me, isd, op=OP.mult)
                nc.vector.tensor_scalar_mul(nb, tp, -1.0)

                o3 = o_sb[:].rearrange("p (t n) -> p t n", n=128)[:, h0:h1, :]
                x3 = x_sb[:].rearrange("p (t n) -> p t n", n=128)[:, h0:h1, :]
                ob3 = obf[:].rearrange("p (t n) -> p t n", n=128)[:, h0:h1, :]
                isd_b = isd.unsqueeze(2).broadcast_to([128, hn, 128])
                nb_b = nb.unsqueeze(2).broadcast_to([128, hn, 128])
                nc.vector.tensor_tensor(o3, o3, isd_b, op=OP.mult)
                nc.vector.tensor_tensor(o3, o3, nb_b, op=OP.add)
                osl = o_sb[:, h0 * 128:h1 * 128]
                nc.scalar.activation(osl, osl, AF.Relu)
                OB = 8
                ob_full = obf[:].rearrange("p (t n) -> p t n", n=128)
                for t0 in range(h0, h1, OB):
                    tb = min(OB, h1 - t0)
                    nc.vector.tensor_tensor(
                        ob_full[:, t0:t0 + tb, :],
                        o_sb[:].rearrange("p (t n) -> p t n", n=128)
                        [:, t0:t0 + tb, :],
                        x_sb[:].rearrange("p (t n) -> p t n", n=128)
                        [:, t0:t0 + tb, :], op=OP.add)
                    dst = out_d.ap()[t0 * 128:(t0 + tb) * 128, :]
                    nc.sync.dma_start(
                        dst.rearrange("(j p) c -> p j c", p=128),
                        ob_full[:, t0:t0 + tb, :])

            if nt >= 16:
                qs = [nt // 4, nt // 2, 3 * nt // 4, nt]
            else:
                qs = [nt]

            # --- phase B: gather + selection matmuls per node tile -----
            # Gathers are flat 8-block (1024-row, the SWDGE ring limit)
            # chunks over the global block list, decoupled from tile
            # boundaries, so no gather instruction runs partially filled.
            CH = 8
            chunk_tiles = {}
            next_chunk = [0]

            def ensure_chunks(upto_blk):
                while next_chunk[0] * CH < upto_blk:
                    k = next_chunk[0]
                    cb = min(CH, nbl_total - k * CH)
                    g = gpool.tile([128, CH, C], bf16, tag="gath")
                    nc.gpsimd.dma_gather(
                        g[:, 0:cb, :],
                        y_dram[:],
                        gidx[:, k * CH * 8:(k * CH + cb) * 8],
                        cb * 128,
                        cb * 128,
                        C,
                        queue_num=k % 2,
                    )
                    chunk_tiles[k] = g
                    next_chunk[0] += 1

            blk0 = 0
            for t in range(nt):
                nblk = nblks[t]
                ps = psb.tile([128, 2 * C], f32, tag="psb")
                nc.tensor.matmul(
                    ps[:],
                    xbt[:, t * 128:(t + 1) * 128],
                    w1uu[:],
                    start=True, stop=(nblk == 0),
                )
                if nblk > 0:
                    ensure_chunks(blk0 + nblk)
                    for j in range(nblk):
                        b = blk0 + j
                        m = mpool.tile([128, 128], bf16, tag="m")
                        nc.vector.tensor_scalar(
                            m[:], colx[:],
                            tgtl[:, b:b + 1],
                            weff[:, b:b + 1],
                            op0=OP.is_equal, op1=OP.mult,
                        )
                        nc.tensor.matmul(
                            ps[:, 0:C], m[:],
                            chunk_tiles[b // CH][:, b % CH, :],
                            start=False, stop=(j == nblk - 1),
                        )
                    blk0 += nblk
                # out tile copy + moments
                nc.scalar.activation(
                    o_sb[:, t * 128:(t + 1) * 128], ps[:, 0:C], AF.Copy,
                    accum_out=stats[:, t:t + 1],
                )
                sq = work.tile([128, C], f32, tag="sq")
                nc.scalar.activation(
                    sq[:], ps[:, 0:C], AF.Square,
                    accum_out=stats[:, nt + t:nt + t + 1],
                )
                for qi in range(len(qs) - 1):
                    if t == qs[qi] + 6:
                        finalize(qs[qi - 1] if qi else 0, qs[qi], qi)

            finalize(qs[-2] if len(qs) > 1 else 0, qs[-1], len(qs) - 1)

    nc.compile()
    return nc


def _get_compiled(key, n_cores, nblks, nbl_total, npad, nt, stats_scale):
    if key not in _cache:
        _cache[key] = build_program(n_cores, nblks, nbl_total, npad, nt,
                                    stats_scale)
    return _cache[key]


LAST_EXEC_NS = None
LAST_RESULT = None
LAST_DISPATCH = None  # (disp, dev_in) of the most recent fast-path call


def _fast_dispatch(nc, n_cores):
    """Build (once) a cached jit'd shard_map dispatcher for the compiled
    Bass module, mirroring bass2jax.run_bass_via_pjrt but with on-device
    output buffers and reusable device-resident inputs."""
    import jax
    import jax.numpy as jnp
    import concourse.mybir as mybir
    from jax.experimental.shard_map import shard_map
    from jax.sharding import Mesh, PartitionSpec
    from concourse import bass2jax

    bass2jax.install_neuronx_cc_hook()

    in_names, out_names, out_avals = [], [], []
    partition_name = (nc.partition_id_tensor.name
                      if nc.partition_id_tensor else None)
    for alloc in nc.m.functions[0].allocations:
        if not isinstance(alloc, mybir.MemoryLocationSet):
            continue
        name = alloc.memorylocations[0].name
        if alloc.kind == "ExternalInput":
            if name != partition_name:
                in_names.append(name)
        elif alloc.kind == "ExternalOutput":
            out_names.append(name)
            out_avals.append(jax.core.ShapedArray(
                tuple(alloc.tensor_shape), mybir.dt.np(alloc.dtype)))
    n_params = len(in_names)
    all_in_names = list(in_names) + list(out_names)
    if partition_name is not None:
        all_in_names.append(partition_name)

    def _body(*args):
        operands = list(args)
        if partition_name is not None:
            operands.append(bass2jax.partition_id_tensor())
        outs = bass2jax._bass_exec_p.bind(
            *operands,
            out_avals=tuple(out_avals),
            in_names=tuple(all_in_names),
            out_names=tuple(out_names),
            lowering_input_output_aliases=(),
            sim_require_finite=True,
            sim_require_nnan=True,
            nc=nc,
        )
        return tuple(outs)

    devices = jax.devices()[:n_cores]
    mesh = Mesh(np.asarray(devices), ("core",))
    n_outs = len(out_avals)
    donate = tuple(range(n_params, n_params + n_outs))
    in_specs = (PartitionSpec("core"),) * (n_params + n_outs)
    out_specs = (PartitionSpec("core"),) * n_outs
    sharded = jax.jit(
        shard_map(_body, mesh=mesh, in_specs=in_specs, out_specs=out_specs,
                  check_rep=False),
        donate_argnums=donate, keep_unused=True)

    from jax.sharding import NamedSharding
    shard = NamedSharding(mesh, PartitionSpec("core"))

    def make_zeros():
        return [jax.device_put(
            jnp.zeros((n_cores * a.shape[0], *a.shape[1:]), a.dtype), shard)
            for a in out_avals]

    return {"sharded": sharded, "in_names": in_names,
            "out_names": out_names, "out_avals": out_avals,
            "shard": shard, "make_zeros": make_zeros, "n_cores": n_cores}


def _run_fast(disp, in_maps):
    global LAST_DISPATCH
    import jax
    import hashlib
    n_cores = disp["n_cores"]
    dev_in = []
    for i, name in enumerate(disp["in_names"]):
        arrs = [np.asarray(m[name]) for m in in_maps]
        h = hashlib.blake2b(digest_size=16)
        h.update(name.encode())
        for a in arrs:
            h.update(a.tobytes())
        key = ("devin", h.hexdigest())
        if key not in _cache:
            cat = np.concatenate(arrs, axis=0)
            _cache[key] = jax.device_put(cat, disp["shard"])
            _cache[key].block_until_ready()
        dev_in.append(_cache[key])
    LAST_DISPATCH = (disp, dev_in)
    zeros = disp["make_zeros"]()
    out_arrs = disp["sharded"](*dev_in, *zeros)
    out_arrs = [np.asarray(o) for o in out_arrs]
    return [
        {name: out_arrs[i].reshape(n_cores, *disp["out_avals"][i].shape)[c]
         for i, name in enumerate(disp["out_names"])}
        for c in range(n_cores)
    ]


def kernel(X, edge_index, edge_weight, weight1, weight2, u, v, _trace=False):
    global LAST_EXEC_NS, LAST_RESULT
    from concourse.bass_utils import run_bass_kernel_spmd

    X = np.asarray(X, dtype=np.float32)
    weight1 = np.asarray(weight1, dtype=np.float32)
    weight2 = np.asarray(weight2, dtype=np.float32)
    u = np.asarray(u, dtype=np.float32)
    v = np.asarray(v, dtype=np.float32)

    ek = (np.asarray(edge_index).tobytes(), np.asarray(edge_weight).tobytes())
    gk = hash(ek)
    if ("graph", gk) not in _cache:
        _cache[("graph", gk)] = _prep_graph(edge_index, edge_weight)
    nblks, gidx, tgtl, weff = _cache[("graph", gk)]
    nbl_total = sum(nblks)

    nc = _get_compiled(("prog", tuple(nblks)), B, nblks, nbl_total, NPAD, NT,
                       1.0 / (B * C))

    import ml_dtypes as mld
    w12 = np.concatenate([weight1, weight1 @ (v * weight2[0][None, :])],
                         axis=1).astype(mld.bfloat16)
    w1uu_half = (weight1 @ u).astype(np.float32)
    w1uu = np.concatenate([w1uu_half, w1uu_half], axis=1).astype(mld.bfloat16)

    xbt = np.zeros((B, C, NPAD), dtype=mld.bfloat16)
    xbt[:, :, :N] = X.transpose(0, 2, 1).astype(mld.bfloat16)

    colx = np.tile(np.arange(C, dtype=np.float32), (128, 1)).astype(mld.bfloat16)
    shared = {"W12": w12, "W1UU": w1uu, "GIDX": gidx, "TGTL": tgtl,
              "WEFF": weff, "COLX": colx}
    in_maps = [{"XBT": np.ascontiguousarray(xbt[i]), **shared}
               for i in range(B)]
    import os
    results = None
    if not _trace and not os.environ.get("KERNEL_NO_FAST"):
        try:
            dk = ("disp", id(nc))
            if dk not in _cache:
                _cache[dk] = _fast_dispatch(nc, B)
            results = _run_fast(_cache[dk], in_maps)
        except Exception:
            results = None
    if results is None:
        res = run_bass_kernel_spmd(nc, in_maps, core_ids=list(range(B)),
                                   trace=_trace)
        LAST_EXEC_NS = res.exec_time_ns
        LAST_RESULT = res
        results = res.results
    out = np.stack([results[i]["OUT"][:N].astype(np.float32)
                    for i in range(B)])
    return out
